# revision 13
# baseline (speedup 1.0000x reference)
"""DIFF-Attention Trainium2 kernel.

Problem: B=2, N=2048, DIM=768, H=12, HD=64, two qkv projections, two
softmax attention maps, diff = attn1 - lam*attn2, out = diff @ v1,
RMSNorm, proj.

Sharding: 8 cores; core c handles batch b = c//4 and query tokens
[512*(c%4), 512*(c%4)+512). Each core computes k1/k2/v1 for its whole
batch (duplicated across the 4 cores of that batch) and q/attention/
norm/proj only for its 512 query tokens. No collectives.

Numerics: float32r (tf32-like) for qkv/QK/proj GEMMs, bf16 for the
exp(S) attention weights and the A@V matmul, fp32 PSUM accumulation
everywhere, RMSNorm in fp32.

A@V orientation: out[q, hd] = E^T @ V_aug with the E tile [keys, q] as
the PE stationary and V_aug [keys, hd+1] as the moving operand -- the
PE streams only 65 rows per 128x128 key/query tile instead of 512, and
the result lands token-major (no transpose needed before RMSNorm).
The +1 column of V_aug is ones, so each map's softmax normalizer
accumulates in column 64 for free.

Schedule: the trace order software-pipelines the head-pair loop --
while pair p's four attention chains (2 heads x 2 attention maps) run,
the k/q GEMMs for pair p+1 are interleaved between chains, and each
chain's A@V matmuls lag one key-chunk behind the QK matmuls so the
scalar-engine exp stays off the critical path.

Layouts (partition dim first):
  xT      [128, 6, 2048]   x[b].T       feature-major (f32r)
  xqT     [128, 6, 512]    query slice of x[b].T (f32r)
  q{1,2}p [128, 512]       per head-pair q^T (f32r, rotating)
  k{1,2}T [128, 2048]      per head-pair k^T (f32r, rotating)
  v1aug   [128, 16, 12, 65] v1 per (tok-tile, head) + ones column (bf16)
  S^T     psum [128, 2, 512] two key-tiles x 512 queries
  E       [128, 2, 512]    exp(S^T/8) (bf16)
  O       psum [128, 4, 65] (E^T @ V_aug) per head, token-major
  Y       [128, 4, 768]    combined attention output, token-major (bf16)
  yT      [128, 6, 512]    Y transposed for proj (bf16)
"""

import numpy as np

B, N, DIM, H, HD = 2, 2048, 768, 12, 64
NQ = 512            # query tokens per core
LAMBDA_INIT = 0.1
EPS = 1e-6
NCORES = 8

_cache = {}
_last_in_maps = None


def _split_waits(nc, max_waits=1):
    """The walrus build in this environment rejects instructions carrying
    more than one explicit sync wait. Hoist excess waits onto NoOps
    inserted just before, on the same engine (same-engine program order
    makes this semantically equivalent)."""
    import concourse.mybir as mybir

    ctr = 0
    for f in nc.m.functions:
        for b in f.blocks:
            out = []
            changed = False
            for inst in b.instructions:
                si = inst.sync_info
                waits = list(si.on_wait) if si is not None and si.on_wait else []
                if len(waits) > max_waits:
                    changed = True
                    keep = waits[-max_waits:]
                    excess = waits[:-max_waits]
                    for i in range(0, len(excess), max_waits):
                        ctr += 1
                        nop = mybir.InstNoOp(
                            name=f"I-waitsplit-{ctr}", ins=[], outs=[]
                        )
                        nop.engine = inst.engine
                        nop.sync_info = mybir.SyncInfo(
                            on_wait=excess[i : i + max_waits], on_update=[]
                        )
                        out.append(nop)
                    inst.sync_info = mybir.SyncInfo(
                        on_wait=keep,
                        on_update=list(si.on_update) if si.on_update else [],
                    )
                out.append(inst)
            if changed:
                b.instructions = out


def _build():
    import concourse.bass as bass
    import concourse.mybir as mybir
    import concourse.tile as tile
    from concourse.masks import make_identity

    f32 = mybir.dt.float32
    f32r = mybir.dt.float32r
    bf16 = mybir.dt.bfloat16

    nc = bass.Bass(trn_type="TRN2")

    xT_d = nc.dram_tensor("xT", [DIM, N], f32r, kind="ExternalInput")
    xqT_d = nc.dram_tensor("xqT", [DIM, NQ], f32r, kind="ExternalInput")
    w1_d = nc.dram_tensor("w1", [DIM, 3 * DIM], f32r, kind="ExternalInput")
    w2_d = nc.dram_tensor("w2", [DIM, 3 * DIM], f32r, kind="ExternalInput")
    wp_d = nc.dram_tensor("wp", [DIM, DIM], bf16, kind="ExternalInput")
    bp_d = nc.dram_tensor("bp", [DIM], f32, kind="ExternalInput")
    lam_d = nc.dram_tensor("lam", [H], f32, kind="ExternalInput")
    out_d = nc.dram_tensor("out", [NQ, DIM], f32, kind="ExternalOutput")

    C = 6          # 768 / 128 feature chunks
    NPAIR = 6      # head pairs
    TT = 16        # token tiles of 128 in N
    QT = 4         # query sub-tiles of 128 in NQ

    with tile.TileContext(nc) as tc:
        with (
            tc.tile_pool(name="persist", bufs=1) as pp,
            tc.tile_pool(name="psum", bufs=1, space="PSUM") as psp,
        ):
            # ---- constants / small tiles ----
            ident = pp.tile([128, 128], bf16, tag="ident")
            make_identity(nc, ident[:])
            lam_b = pp.tile([128, H], f32, tag="lam_b")
            nc.gpsimd.dma_start(
                out=lam_b[:],
                in_=bass.AP(tensor=lam_d, offset=0, ap=[[0, 128], [1, H]]),
            )
            bp_b = pp.tile([128, DIM], f32, tag="bp_b")
            nc.gpsimd.dma_start(
                out=bp_b[:],
                in_=bass.AP(tensor=bp_d, offset=0, ap=[[0, 128], [1, DIM]]),
            )

            # ---- resident big tiles; xT arrives in 4 token-slices ----
            xqT = pp.tile([128, C, NQ], f32r, tag="xqT")
            for cch in range(C):
                nc.sync.dma_start(
                    xqT[:, cch, :],
                    xqT_d[cch * 128 : (cch + 1) * 128, :],
                )
            xT = pp.tile([128, C, N], f32r, tag="xT")

            def dma_xT():
                for s in range(4):
                    nc.sync.dma_start(
                        xT[:, :, s * 512 : (s + 1) * 512],
                        xT_d[:, s * 512 : (s + 1) * 512].rearrange(
                            "(c p) m -> p c m", p=128
                        ),
                    )

            v1aug = pp.tile([128, TT, H, HD + 1], bf16, tag="v1aug")
            nc.vector.memset(v1aug[:, :, :, HD : HD + 1], 1.0)
            Y = pp.tile([128, QT, DIM], bf16, tag="Y")
            yT = pp.tile([128, C, NQ], bf16, tag="yT")
            stats = pp.tile([128, QT, C, 6], f32, tag="stats")
            wpj = pp.tile([128, C, DIM], bf16, tag="wbig2")

            with (
                tc.tile_pool(name="phaseA", bufs=1) as pa,
                tc.tile_pool(name="pairs", bufs=2) as wpool,
                tc.tile_pool(name="epool", bufs=3) as ep,
            ):
                # ---- weight slice DMA + GEMM emit helpers ----
                def dma_wslice(tag, src_w, col0):
                    t = wpool.tile([128, C, 128], f32r, tag=tag, name=tag)
                    nc.sync.dma_start(
                        t[:],
                        src_w[:, col0 : col0 + 128].rearrange(
                            "(c p2) n -> p2 c n", p2=128
                        ),
                    )
                    return t

                def emit_q_gemm(wq, tag):
                    qp = wpool.tile([128, NQ], f32r, tag=tag, name=tag)
                    ps = psp.tile([128, NQ], f32, tag="mm", bufs=2, name="psq")
                    for c in range(C):
                        nc.tensor.matmul(
                            ps[:],
                            wq[:, c, :],
                            xqT[:, c, :],
                            start=(c == 0),
                            stop=(c == C - 1),
                        )
                    nc.vector.tensor_copy(qp[:], ps[:])
                    return qp

                def alloc_k(tag):
                    return wpool.tile([128, N], f32r, tag=tag, name=tag)

                def emit_k_gemm(kt, wk, mt):
                    ps = psp.tile([128, 512], f32, tag="mm", bufs=2, name="psk")
                    for c in range(C):
                        nc.tensor.matmul(
                            ps[:],
                            wk[:, c, :],
                            xT[:, c, mt * 512 : (mt + 1) * 512],
                            start=(c == 0),
                            stop=(c == C - 1),
                        )
                    nc.vector.tensor_copy(kt[:, mt * 512 : (mt + 1) * 512], ps[:])

                # ---- pair-0 weights + GEMMs; v1 via prefetch queue ----
                wq1 = dma_wslice("wq1", w1_d, 0)
                wq2 = dma_wslice("wq2", w2_d, 0)
                wk1 = dma_wslice("wk1", w1_d, DIM)
                wk2 = dma_wslice("wk2", w2_d, DIM)
                dma_xT()
                wv1 = pa.tile([128, C, DIM], f32r, tag="wbig")
                nc.sync.dma_start(
                    wv1[:],
                    w1_d[:, 2 * DIM : 3 * DIM].rearrange(
                        "(c p) n -> p c n", p=128
                    ),
                )
                q1p = emit_q_gemm(wq1, "q1p")
                q2p = emit_q_gemm(wq2, "q2p")
                k1T = alloc_k("k1T")
                k2T = alloc_k("k2T")
                for mt in range(4):
                    emit_k_gemm(k1T, wk1, mt)
                for mt in range(4):
                    emit_k_gemm(k2T, wk2, mt)

                def emit_v1_tile(t):
                    for half in range(2):
                        ps = psp.tile([128, 384], f32, tag="mm", bufs=2, name="psv")
                        for c in range(C):
                            nc.tensor.matmul(
                                ps[:],
                                xT[:, c, t * 128 : (t + 1) * 128],
                                wv1[:, c, half * 384 : (half + 1) * 384],
                                start=(c == 0),
                                stop=(c == C - 1),
                            )
                        nc.vector.tensor_copy(
                            v1aug[:, t, 6 * half : 6 * half + 6, 0:HD],
                            ps[:].rearrange("p (h d) -> p h d", h=6),
                        )

                from collections import deque

                for t in range(4):
                    emit_v1_tile(t)
                v1_q = deque(range(4, TT))

                def v1_hook(g):
                    # keep v1 tile production two AV groups ahead
                    for _ in range(2):
                        if v1_q:
                            emit_v1_tile(v1_q.popleft())

                work_q = deque()

                def pop_work(n):
                    for _ in range(n):
                        if work_q:
                            work_q.popleft()()

                # ---- pair loop, software-pipelined ----
                def attn_chain(h, po, kt, qp, group_hook=None):
                    """One head x one attention map: accumulates
                    O = E^T @ V_aug into a [128, QT, 65] psum tile
                    (token-major; col 64 = softmax normalizer)."""
                    # padded to [.., 128] so each buffer is exactly one
                    # 2KB PSUM bank (matmul outputs cannot straddle banks).
                    # The four q-subtile accumulation groups share that bank,
                    # and start_tensor_calc zeroes the whole bank -- so zero
                    # it once up front and accumulate with start=False.
                    av = psp.tile(
                        [128, QT, 128], f32, tag="av", bufs=2, name="av"
                    )
                    nc.vector.memset(av[:], 0.0)

                    def emit_av(e_t, g):
                        for g2 in range(2):
                            mc = g * 2 + g2
                            for j in range(QT):
                                nc.tensor.matmul(
                                    av[:, j, 0 : HD + 1],
                                    e_t[:, g2, j * 128 : (j + 1) * 128],
                                    v1aug[:, mc, h, :],
                                    start=False,
                                    stop=(mc == 15),
                                    skip_group_check=True,
                                )

                    pend = deque()
                    for g in range(8):
                        if group_hook is not None:
                            group_hook(g)
                        qk = psp.tile(
                            [128, 2, 512], f32, tag="qk", bufs=2, name="qk"
                        )
                        for g2 in range(2):
                            mc = g * 2 + g2
                            nc.tensor.matmul(
                                qk[:, g2, :],
                                kt[po : po + 64, mc * 128 : (mc + 1) * 128],
                                qp[po : po + 64, :],
                                start=True,
                                stop=True,
                            )
                        e_t = ep.tile([128, 2, 512], bf16, tag="E", name="e_t")
                        nc.scalar.activation(
                            e_t[:],
                            qk[:],
                            mybir.ActivationFunctionType.Exp,
                            scale=0.125,
                        )
                        pend.append((e_t, g))
                        # A@V lags one key-chunk so exp stays off the PE
                        # critical path
                        if len(pend) > 1:
                            emit_av(*pend.popleft())
                    while pend:
                        emit_av(*pend.popleft())
                    return av

                def combine(h, av1, av2):
                    r1 = wpool.tile([128, QT, 1], f32, tag="r1", bufs=2, name="r1")
                    nc.vector.reciprocal(r1[:], av1[:, :, HD : HD + 1])
                    r2 = wpool.tile([128, QT, 1], f32, tag="r2", bufs=2, name="r2")
                    nc.vector.reciprocal(r2[:], av2[:, :, HD : HD + 1])
                    lam_h = lam_b[:, h : h + 1]
                    lam_bc = bass.AP(
                        tensor=lam_h.tensor,
                        offset=lam_h.offset,
                        ap=[lam_h.ap[0], [0, QT], [0, 1]],
                    )
                    nc.vector.tensor_tensor(
                        out=r2[:], in0=r2[:], in1=lam_bc, op=mybir.AluOpType.mult
                    )
                    t1 = wpool.tile([128, QT, HD], f32, tag="t1", bufs=1, name="t1")
                    t2 = wpool.tile([128, QT, HD], f32, tag="t2", bufs=1, name="t2")
                    for r, src, dst in ((r1, av1, t1), (r2, av2, t2)):
                        rb = bass.AP(
                            tensor=r.tensor,
                            offset=r.offset,
                            ap=[r.ap[0], r.ap[1], [0, HD]],
                        )
                        nc.vector.tensor_tensor(
                            out=dst[:],
                            in0=src[:, :, 0:HD],
                            in1=rb,
                            op=mybir.AluOpType.mult,
                        )
                    nc.vector.tensor_tensor(
                        out=Y[:, :, h * 64 : (h + 1) * 64],
                        in0=t1[:],
                        in1=t2[:],
                        op=mybir.AluOpType.subtract,
                    )

                nxt_state = {}
                for p in range(NPAIR):
                    nxt = p + 1
                    if nxt < NPAIR:
                        wk1n = dma_wslice("wk1", w1_d, DIM + nxt * 128)
                        wk2n = dma_wslice("wk2", w2_d, DIM + nxt * 128)
                        wq1n = dma_wslice("wq1", w1_d, nxt * 128)
                        wq2n = dma_wslice("wq2", w2_d, nxt * 128)
                        k1Tn = alloc_k("k1T")
                        k2Tn = alloc_k("k2T")
                        nxt_state.clear()
                        work_q.append(
                            lambda w=wq1n: nxt_state.__setitem__(
                                "q1p", emit_q_gemm(w, "q1p")
                            )
                        )
                        for mt in range(4):
                            work_q.append(
                                lambda kt=k1Tn, w=wk1n, m=mt: emit_k_gemm(kt, w, m)
                            )
                        work_q.append(
                            lambda w=wq2n: nxt_state.__setitem__(
                                "q2p", emit_q_gemm(w, "q2p")
                            )
                        )
                        for mt in range(4):
                            work_q.append(
                                lambda kt=k2Tn, w=wk2n, m=mt: emit_k_gemm(kt, w, m)
                            )

                    # 4 chains; prefetch work spread evenly between chains
                    av1 = attn_chain(
                        2 * p, 0, k1T, q1p, group_hook=v1_hook if p == 0 else None
                    )
                    pop_work(-(-len(work_q) // 4))
                    av2 = attn_chain(2 * p, 0, k2T, q2p)
                    pop_work(-(-len(work_q) // 3))
                    combine(2 * p, av1, av2)
                    av1 = attn_chain(2 * p + 1, 64, k1T, q1p)
                    pop_work(-(-len(work_q) // 2))
                    av2 = attn_chain(2 * p + 1, 64, k2T, q2p)
                    pop_work(len(work_q))
                    combine(2 * p + 1, av1, av2)

                    # Y chunk p (heads 2p,2p+1 = cols 128p..128p+128) is now
                    # final: transpose + norm-stats overlap the next pair.
                    for j in range(QT):
                        ptr = psp.tile(
                            [128, 128], bf16, tag="mm", bufs=2, name="ptr"
                        )
                        nc.tensor.transpose(
                            ptr[:], Y[:, j, p * 128 : (p + 1) * 128], ident[:]
                        )
                        nc.vector.tensor_copy(
                            yT[:, p, j * 128 : (j + 1) * 128], ptr[:]
                        )
                        nc.vector.bn_stats(
                            out=stats[:, j, p, :],
                            in_=Y[:, j, p * 128 : (p + 1) * 128],
                        )

                    if p == 2:
                        # DMA engines are idle mid-attention; prefetch the
                        # (bf16) proj weights now so the tail never waits.
                        nc.sync.dma_start(
                            wpj[:],
                            wp_d[:, :].rearrange("(c p) n -> p c n", p=128),
                        )

                    if nxt < NPAIR:
                        while work_q:
                            work_q.popleft()()
                        k1T, k2T = k1Tn, k2Tn
                        q1p, q2p = nxt_state["q1p"], nxt_state["q2p"]

            # ---- tail: finish RMSNorm in feature-major orientation, proj ----
            # (norm_w folded into wp host-side)
            with tc.tile_pool(name="proj", bufs=1) as prj:
                mv = prj.tile([128, 2], f32, tag="mv")
                rms = prj.tile([128, QT], f32, tag="rms")
                eps_t = prj.tile([128, 1], f32, tag="eps_t")
                nc.vector.memset(eps_t[:], EPS)
                for j in range(QT):
                    nc.vector.bn_aggr(out=mv[:], in_=stats[:, j])
                    # E[y^2] = var + mean^2
                    nc.vector.tensor_tensor(
                        out=mv[:, 0:1],
                        in0=mv[:, 0:1],
                        in1=mv[:, 0:1],
                        op=mybir.AluOpType.mult,
                    )
                    nc.vector.tensor_tensor(
                        out=mv[:, 1:2],
                        in0=mv[:, 1:2],
                        in1=mv[:, 0:1],
                        op=mybir.AluOpType.add,
                    )
                    nc.scalar.activation(
                        rms[:, j : j + 1],
                        mv[:, 1:2],
                        mybir.ActivationFunctionType.Sqrt,
                        bias=eps_t[:],
                        scale=1.0,
                    )
                    nc.vector.reciprocal(rms[:, j : j + 1], rms[:, j : j + 1])
                # proj on the UNNORMALIZED yT; rms folds into the epilogue:
                # (y*rms) @ Wp = (y @ Wp) * rms  (rms is per-token = per out row)
                for j in range(QT):
                    jr = slice(j * 128, (j + 1) * 128)
                    osb2 = prj.tile([128, DIM], f32, tag="out_sb", bufs=2, name="osb2")
                    for half in range(2):
                        ps = psp.tile([128, 384], f32, tag="mm", bufs=2, name="psp2")
                        for c in range(C):
                            nc.tensor.matmul(
                                ps[:],
                                yT[:, c, jr],
                                wpj[:, c, half * 384 : (half + 1) * 384],
                                start=(c == 0),
                                stop=(c == C - 1),
                            )
                        hs = slice(half * 384, (half + 1) * 384)
                        nc.vector.tensor_scalar_mul(
                            osb2[:, hs], ps[:], rms[:, j : j + 1]
                        )
                        nc.vector.tensor_tensor(
                            out=osb2[:, hs],
                            in0=osb2[:, hs],
                            in1=bp_b[:, hs],
                            op=mybir.AluOpType.add,
                        )
                    nc.sync.dma_start(
                        out_d[j * 128 : (j + 1) * 128, :], osb2[:]
                    )

    _split_waits(nc)
    return nc


def kernel(x, W_qkv1, W_qkv2, W_proj, b_proj, norm_w, lambda_1, lambda_2, xpos):
    from concourse.bass_utils import run_bass_kernel_spmd

    if "nc" not in _cache:
        _cache["nc"] = _build()
    nc = _cache["nc"]

    x = np.asarray(x, dtype=np.float32)
    w1 = np.ascontiguousarray(np.asarray(W_qkv1, dtype=np.float32))
    w2 = np.ascontiguousarray(np.asarray(W_qkv2, dtype=np.float32))
    import ml_dtypes

    wp = np.ascontiguousarray(
        (
            np.asarray(norm_w, dtype=np.float32)[:, None]
            * np.asarray(W_proj, dtype=np.float32)
        ).astype(ml_dtypes.bfloat16)
    )
    bp = np.ascontiguousarray(np.asarray(b_proj, dtype=np.float32))
    lam = np.ascontiguousarray(
        (
            np.asarray(lambda_1, dtype=np.float32)
            - np.asarray(lambda_2, dtype=np.float32)
            + LAMBDA_INIT
        ).astype(np.float32)
    )

    xTs = [np.ascontiguousarray(x[b].T) for b in range(B)]
    in_maps = []
    for c in range(NCORES):
        b, qi = c // 4, c % 4
        in_maps.append(
            {
                "xT": xTs[b],
                "xqT": np.ascontiguousarray(xTs[b][:, qi * NQ : (qi + 1) * NQ]),
                "w1": w1,
                "w2": w2,
                "wp": wp,
                "bp": bp,
                "lam": lam,
            }
        )

    global _last_in_maps
    _last_in_maps = in_maps
    res = run_bass_kernel_spmd(nc, in_maps, core_ids=list(range(NCORES)))
    out = np.empty((B, N, DIM), dtype=np.float32)
    for c in range(NCORES):
        b, qi = c // 4, c % 4
        out[b, qi * NQ : (qi + 1) * NQ, :] = res.results[c]["out"]
    return out
